# revision 1
# baseline (speedup 1.0000x reference)
"""Parametric Bass/Tile attention-layer kernel for TRN2 (8-core data parallel).

Per-core computation (BP batch elements each):
  h      = (x @ w_in.T + b_in + te) * scale          [T, E]
  scores = h @ keys + (-1e9 * mask)                  [T, S]
  attn   = softmax(scores, axis=-1)                  -> output
  ctx    = (attn @ values) * sqrt(valid)             [T, E]
  out    = (ctx @ w_out.T + b_out + x) * scale       -> output

All big matmuls run as float32r (full PE rate at free>=256); rank-1 bias /
mask / residual folds also f32r. Transposes via PE with identity. Scale `s`
applied on the ACT psum->sbuf copies (free); sqrt(valid) applied on the ctx
copy via per-partition ACT scale.

Software pipelining: x-load + x^T of block n+1 are emitted before the
attn^T/mm3/mm4 tail of block n so the PE fills the softmax-latency bubble.
"""

import math
import os
import sys
import tempfile

os.environ.setdefault("NEURON_COMPILE_CACHE_URL",
                      tempfile.mkdtemp(prefix="neuroncache_"))

sys.path.insert(0, "/opt/trn_rl_repo")
sys.path.insert(0, "/opt/trn_rl_repo/concourse")

from contextlib import ExitStack

import concourse.bass as bass
import concourse.tile as tile
from concourse import bacc, mybir

P = 128
f32 = mybir.dt.float32
f32r = mybir.dt.float32r
u8 = mybir.dt.uint8
AF = mybir.ActivationFunctionType
ALU = mybir.AluOpType

NEG_BIG = -1.0e9


def build_attn(n_cores=8, BP=2, T=1024, S=1024, C=1024, E=1024, TBLK=256,
               scale=math.sqrt(0.5), psum_bufs=8):
    CT, ET, ST = C // P, E // P, S // P
    NB = T // TBLK
    TPB = TBLK // P
    SN = min(256, S)
    CN = min(256, C)

    nc = bacc.Bacc("TRN2", target_bir_lowering=False, debug=False,
                   num_devices=n_cores)

    x_d = nc.dram_tensor("x", [BP, T, C], f32, kind="ExternalInput").ap()
    te_d = nc.dram_tensor("te", [BP, T, E], f32, kind="ExternalInput").ap()
    k_d = nc.dram_tensor("keys", [BP, E, S], f32, kind="ExternalInput").ap()
    v_d = nc.dram_tensor("values", [BP, S, E], f32, kind="ExternalInput").ap()
    m_d = nc.dram_tensor("mask", [BP, S], u8, kind="ExternalInput").ap()
    wi_d = nc.dram_tensor("w_in", [E, C], f32, kind="ExternalInput").ap()
    bi_d = nc.dram_tensor("b_in", [1, E], f32, kind="ExternalInput").ap()
    wo_d = nc.dram_tensor("w_out", [C, E], f32, kind="ExternalInput").ap()
    bo_d = nc.dram_tensor("b_out", [1, C], f32, kind="ExternalInput").ap()
    out_d = nc.dram_tensor("out", [BP, T, C], f32, kind="ExternalOutput").ap()
    attn_d = nc.dram_tensor("attn", [BP, T, S], f32, kind="ExternalOutput").ap()

    with tile.TileContext(nc) as tc, ExitStack() as ctx:
        consts = ctx.enter_context(tc.tile_pool(name="consts", bufs=1))
        batchp = ctx.enter_context(tc.tile_pool(name="batchp", bufs=1))
        blocks = ctx.enter_context(tc.tile_pool(name="blocks", bufs=1))
        tiles = ctx.enter_context(tc.tile_pool(name="tiles", bufs=2))
        stats = ctx.enter_context(tc.tile_pool(name="stats", bufs=2))
        psum = ctx.enter_context(
            tc.tile_pool(name="psum", bufs=psum_bufs, space="PSUM"))

        _ps_ctr = [0]

        def ps_tile(w, dt=f32):
            _ps_ctr[0] += 1
            return psum.tile([P, w], dt, tag="ps", bufs=psum_bufs,
                             name=f"ps{_ps_ctr[0]}")

        # ---- constants ----
        ident = consts.tile([P, P], f32, tag="ident")
        nc.gpsimd.memset(ident, 0.0)
        nc.gpsimd.affine_select(out=ident, in_=ident,
                                compare_op=ALU.not_equal, fill=1.0,
                                base=0, pattern=[[-1, P]], channel_multiplier=1)
        ident_r = consts.tile([P, P], f32r, tag="ident_r")
        nc.vector.tensor_copy(ident_r[:], ident[:])
        ident_s = consts.tile([P, P], f32r, tag="ident_s")
        nc.scalar.activation(ident_s[:], ident[:], AF.Copy, scale=scale)

        # b_in as [P, ET] column tile: sbi[p, et] = scale * b_in[et*P + p]
        sbi = consts.tile([P, ET], f32, tag="sbi")
        bi_cols = bass.AP(tensor=bi_d.tensor, offset=bi_d.offset,
                          ap=[[1, P], [P, ET]])
        nc.sync.dma_start(sbi[:], bi_cols)
        nc.vector.tensor_scalar_mul(sbi[:], sbi[:], scale)
        bo_rep = consts.tile([P, C], f32, tag="bo_rep")
        nc.sync.dma_start(bo_rep[0:1, :], bo_d)
        nc.vector.tensor_scalar_mul(bo_rep[0:1, :], bo_rep[0:1, :], scale)
        nc.gpsimd.partition_broadcast(bo_rep[:], bo_rep[0:1, :])

        # ---- weight prep: wiT[ct] = w_in.T c-tile [P, E] f32r ----
        wiT = [consts.tile([P, E], f32r, tag=f"wiT{i}", name=f"wiT{i}")
               for i in range(CT)]
        woT = [consts.tile([P, C], f32r, tag=f"woT{i}", name=f"woT{i}")
               for i in range(ET)]

        def prep_weights(w_nat_dram, n_in_tiles, n_out_tiles, out_tiles):
            chunk = min(512, n_in_tiles * P)   # psum column chunk
            per = chunk // P
            lhalf = min(512, n_out_tiles * P)  # load half-width
            nhalf = (n_out_tiles * P) // lhalf
            for ech in range(n_in_tiles // per):
                pss = [psum.tile([P, chunk], f32, tag="ps", bufs=psum_bufs,
                                 name=f"wps{ot}") for ot in range(n_out_tiles)]
                for j in range(per):
                    it = ech * per + j
                    for hf in range(nhalf):
                        wn = tiles.tile([P, lhalf], f32, tag="wnat", bufs=3,
                                        name="wn")
                        nc.scalar.dma_start(
                            wn[:], w_nat_dram[it * P:(it + 1) * P,
                                              hf * lhalf:(hf + 1) * lhalf])
                        for oi in range(lhalf // P):
                            ot = hf * (lhalf // P) + oi
                            nc.tensor.matmul(
                                pss[ot][:, j * P:(j + 1) * P],
                                wn[:, oi * P:(oi + 1) * P], ident[:],
                                is_transpose=True, start=(j == 0),
                                stop=(j == per - 1), skip_group_check=True)
                for ot in range(n_out_tiles):
                    nc.vector.tensor_copy(
                        out_tiles[ot][:, ech * chunk:(ech + 1) * chunk],
                        pss[ot][:])



        # ---- per-batch state ----
        state = {}

        def batch_prep_a(b):
            keys_r = [batchp.tile([P, S], f32r, tag=f"keys{i}",
                                  name=f"keys{i}") for i in range(ET)]
            for et in range(ET):
                nc.gpsimd.dma_start(keys_r[et][:],
                                    k_d[b, et * P:(et + 1) * P, :])
            state[b] = [keys_r, None, None, None]

            m8 = batchp.tile([1, S], u8, tag="m8")
            nc.sync.dma_start(m8[:], m_d[b:b + 1, :])
            maskrep = batchp.tile([P, S], f32, tag="maskrep")
            nc.vector.tensor_scalar_mul(maskrep[0:1, :], m8[:], NEG_BIG)
            nvalid = batchp.tile([1, 1], f32, tag="nvalid")
            nc.vector.tensor_reduce(nvalid[:], maskrep[0:1, :],
                                    axis=mybir.AxisListType.X, op=ALU.add)
            nc.gpsimd.partition_broadcast(maskrep[:], maskrep[0:1, :])
            nc.vector.tensor_scalar(nvalid[:], nvalid[:], 1.0 / 1.0e9,
                                    float(S), op0=ALU.mult, op1=ALU.add)
            nc.scalar.activation(nvalid[:], nvalid[:], AF.Ln)
            nc.vector.tensor_scalar_mul(nvalid[:], nvalid[:], 0.5)
            nc.scalar.activation(nvalid[:], nvalid[:], AF.Exp)
            nc.vector.tensor_scalar_mul(nvalid[:], nvalid[:], scale)
            ssv_rep = batchp.tile([P, 1], f32, tag="ssv_rep")
            nc.gpsimd.partition_broadcast(ssv_rep[:], nvalid[:])
            state[b][2] = maskrep
            state[b][3] = ssv_rep

        def batch_prep_b(b):
            vals_r = [batchp.tile([P, E], f32r, tag=f"vals{i}",
                                  name=f"vals{i}") for i in range(ST)]
            for st in range(ST):
                nc.gpsimd.dma_start(vals_r[st][:],
                                    v_d[b, st * P:(st + 1) * P, :])
            state[b][1] = vals_r

        def stage_x(b, blk):
            t0 = blk * TBLK
            x_t = [tiles.tile([P, C], f32r, tag="x", bufs=2 * TPB,
                              name=f"x{i}") for i in range(TPB)]
            for tt in range(TPB):
                nc.gpsimd.dma_start(
                    x_t[tt][:], x_d[b, t0 + tt * P:t0 + (tt + 1) * P, :])
            xT = blocks.tile([P, CT, TBLK], f32r, tag="xT")
            for ct in range(CT):
                ps = ps_tile(TBLK, f32r)
                for tt in range(TPB):
                    nc.tensor.matmul(ps[:, tt * P:(tt + 1) * P],
                                     x_t[tt][:, ct * P:(ct + 1) * P],
                                     ident_r[:], is_transpose=True,
                                     start=(tt == 0), stop=(tt == TPB - 1),
                                     skip_group_check=True)
                nc.vector.tensor_copy(xT[:, ct, :], ps[:])
            return x_t, xT

        def stage_mm1(b, blk, xT):
            t0 = blk * TBLK
            te_t = [tiles.tile([P, E], f32, tag="te", bufs=TPB,
                               name=f"te{i}") for i in range(TPB)]
            for tt in range(TPB):
                nc.sync.dma_start(
                    te_t[tt][:], te_d[b, t0 + tt * P:t0 + (tt + 1) * P, :])
            hT = blocks.tile([P, ET, TBLK], f32r, tag="blkB")
            for et in range(ET):
                ps = ps_tile(TBLK)
                for tt in range(TPB):
                    nc.tensor.matmul(ps[:, tt * P:(tt + 1) * P],
                                     te_t[tt][:, et * P:(et + 1) * P],
                                     ident[:], is_transpose=True,
                                     start=(tt == 0), stop=False,
                                     skip_group_check=True)
                for ct in range(CT):
                    nc.tensor.matmul(ps[:], wiT[ct][:, et * P:(et + 1) * P],
                                     xT[:, ct, :], start=False,
                                     stop=(ct == CT - 1),
                                     skip_group_check=True)
                nc.scalar.activation(hT[:, et, :], ps[:], AF.Identity,
                                     scale=scale, bias=sbi[:, et:et + 1])
            return hT

        def stage_mm2(b, blk, hT):
            keys_r, _, maskrep, _ = state[b]
            t0 = blk * TBLK
            sc_t = []
            for tt in range(TPB):
                sc = tiles.tile([P, S], f32, tag="sc", bufs=TPB, name="sc")
                sc_t.append(sc)
                # per-chunk raw negmax from psum, in parallel with ACT copies.
                # (raw max >= masked max, which is all softmax stability needs;
                # masked cols get -1e9 before exp and underflow to exactly 0.)
                nmx = stats.tile([P, S // SN], f32, tag="nmx", bufs=4)
                for sch in range(S // SN):
                    ps = ps_tile(SN)
                    for et in range(ET):
                        nc.tensor.matmul(
                            ps[:], hT[:, et, tt * P:(tt + 1) * P],
                            keys_r[et][:, sch * SN:(sch + 1) * SN],
                            start=(et == 0), stop=(et == ET - 1),
                            skip_group_check=True)
                    nc.scalar.copy(sc[:, sch * SN:(sch + 1) * SN], ps[:])
                    nc.vector.tensor_reduce(nmx[:, sch:sch + 1], ps[:],
                                            axis=mybir.AxisListType.X,
                                            op=ALU.max, negate=True)
                nc.vector.tensor_tensor(out=sc[:], in0=sc[:], in1=maskrep[:],
                                        op=ALU.add)
                negmax = stats.tile([P, 1], f32, tag="negmax")
                nc.vector.tensor_reduce(negmax[:], nmx[:],
                                        axis=mybir.AxisListType.X,
                                        op=ALU.min)
                sumexp = stats.tile([P, 1], f32, tag="sumexp")
                nc.scalar.activation(sc[:], sc[:], AF.Exp,
                                     bias=negmax[:, 0:1], scale=1.0,
                                     accum_out=sumexp[:, 0:1])
                recip = stats.tile([P, 1], f32, tag="recip")
                nc.vector.reciprocal(recip[:], sumexp[:])
                nc.vector.tensor_scalar_mul(sc[:], sc[:], recip[:, 0:1])
                nc.sync.dma_start(
                    attn_d[b, t0 + tt * P:t0 + (tt + 1) * P, :], sc[:])
            return sc_t

        def stage_tail(b, blk, sc_t, x_t):
            _, vals_r, _, ssv_rep = state[b]
            t0 = blk * TBLK
            aT = blocks.tile([P, ST, TBLK], f32r, tag="blkB")
            for st in range(ST):
                ps = ps_tile(TBLK)
                for tt in range(TPB):
                    nc.tensor.matmul(ps[:, tt * P:(tt + 1) * P],
                                     sc_t[tt][:, st * P:(st + 1) * P],
                                     ident[:], is_transpose=True,
                                     start=(tt == 0), stop=(tt == TPB - 1),
                                     skip_group_check=True)
                nc.vector.tensor_copy(aT[:, st, :], ps[:])

            cxT = blocks.tile([P, ET, TBLK], f32r, tag="cxT")
            for et in range(ET):
                ps = ps_tile(TBLK)
                for st in range(ST):
                    nc.tensor.matmul(ps[:], vals_r[st][:, et * P:(et + 1) * P],
                                     aT[:, st, :], start=(st == 0),
                                     stop=(st == ST - 1))
                nc.scalar.activation(cxT[:, et, :], ps[:], AF.Copy,
                                     scale=ssv_rep[:, 0:1])

            for tt in range(TPB):
                ot = tiles.tile([P, C], f32, tag="ot", bufs=1, name="ot")
                for cch in range(C // CN):
                    ps = ps_tile(CN)
                    nc.tensor.matmul(ps[:], ident_s[:],
                                     x_t[tt][:, cch * CN:(cch + 1) * CN],
                                     start=True, stop=False,
                                     skip_group_check=True)
                    for et in range(ET):
                        nc.tensor.matmul(
                            ps[:], cxT[:, et, tt * P:(tt + 1) * P],
                            woT[et][:, cch * CN:(cch + 1) * CN],
                            start=False, stop=(et == ET - 1),
                            skip_group_check=True)
                    nc.vector.tensor_tensor(
                        out=ot[:, cch * CN:(cch + 1) * CN], in0=ps[:],
                        in1=bo_rep[:, cch * CN:(cch + 1) * CN], op=ALU.add)
                nc.sync.dma_start(
                    out_d[b, t0 + tt * P:t0 + (tt + 1) * P, :], ot[:])

        # ---- pipelined emission over (batch, block) ----
        seq = [(b, blk) for b in range(BP) for blk in range(NB)]
        prep_weights(wi_d, ET, CT, wiT)
        cur = stage_x(*seq[0])
        batch_prep_a(0)
        prep_weights(wo_d, CT, ET, woT)
        batch_prep_b(0)
        for i, (b, blk) in enumerate(seq):
            hT = stage_mm1(b, blk, cur[1])
            sc = stage_mm2(b, blk, hT)
            nxt = None
            if i + 1 < len(seq):
                nb, nblk = seq[i + 1]
                if nb != b:
                    batch_prep_a(nb)
                    batch_prep_b(nb)
                nxt = stage_x(nb, nblk)
            stage_tail(b, blk, sc, cur[0])
            cur = nxt

    nc.compile()
    return nc

N_CORES = 8
B, T, S, C, E = 16, 1024, 1024, 1024, 1024
BP = B // N_CORES

_NC = None
_RUNNER = None


def _make_runner(nc):
    """Reusable jitted 8-core runner (modeled on
    concourse.bass2jax.run_bass_via_pjrt, cached across calls)."""
    import jax
    import numpy as np
    from jax.sharding import Mesh, PartitionSpec
    from jax.experimental.shard_map import shard_map
    from concourse.bass2jax import (_bass_exec_p, install_neuronx_cc_hook,
                                    partition_id_tensor)

    install_neuronx_cc_hook()
    partition_name = nc.partition_id_tensor.name if nc.partition_id_tensor else None

    in_names, out_names, out_avals, zero_shapes = [], [], [], []
    for alloc in nc.m.functions[0].allocations:
        if not isinstance(alloc, mybir.MemoryLocationSet):
            continue
        name = alloc.memorylocations[0].name
        if alloc.kind == "ExternalInput":
            if name != partition_name:
                in_names.append(name)
        elif alloc.kind == "ExternalOutput":
            shape = tuple(alloc.tensor_shape)
            dtype = mybir.dt.np(alloc.dtype)
            out_names.append(name)
            out_avals.append(jax.core.ShapedArray(shape, dtype))
            zero_shapes.append((shape, dtype))
    n_params = len(in_names)
    all_in_names = list(in_names) + list(out_names)
    if partition_name is not None:
        all_in_names.append(partition_name)

    def _body(*args):
        operands = list(args)
        if partition_name is not None:
            operands.append(partition_id_tensor())
        outs = _bass_exec_p.bind(
            *operands, out_avals=tuple(out_avals), in_names=tuple(all_in_names),
            out_names=tuple(out_names), lowering_input_output_aliases=(),
            sim_require_finite=True, sim_require_nnan=True, nc=nc)
        return tuple(outs)

    devices = jax.devices()[:N_CORES]
    mesh = Mesh(np.asarray(devices), ("core",))
    n_outs = len(out_names)
    sharded = jax.jit(
        shard_map(_body, mesh=mesh,
                  in_specs=(PartitionSpec("core"),) * (n_params + n_outs),
                  out_specs=(PartitionSpec("core"),) * n_outs,
                  check_rep=False),
        keep_unused=True)
    zeros = [np.zeros((N_CORES * s[0], *s[1:]), d) for s, d in zero_shapes]

    def run(in_maps):
        concat_in = [
            np.concatenate([np.asarray(m[name]) for m in in_maps], axis=0)
            for name in in_names
        ]
        out_arrs = sharded(*concat_in, *zeros)
        jax.block_until_ready(out_arrs)
        return {name: np.asarray(out_arrs[i]) for i, name in enumerate(out_names)}

    return run


def kernel(x, target_embedding, enc_keys, enc_values, encoder_padding_mask,
           w_in, b_in, w_out, b_out):
    import numpy as np
    global _NC, _RUNNER
    if _NC is None:
        _NC = build_attn(n_cores=N_CORES, BP=BP, T=T, S=S, C=C, E=E, TBLK=256)
        _RUNNER = _make_runner(_NC)

    x = np.ascontiguousarray(np.asarray(x, dtype=np.float32))
    te = np.ascontiguousarray(np.asarray(target_embedding, dtype=np.float32))
    keys = np.ascontiguousarray(np.asarray(enc_keys, dtype=np.float32))
    values = np.ascontiguousarray(np.asarray(enc_values, dtype=np.float32))
    mask = np.ascontiguousarray(np.asarray(encoder_padding_mask)).astype(np.uint8)
    w_in = np.ascontiguousarray(np.asarray(w_in, dtype=np.float32))
    b_in = np.ascontiguousarray(np.asarray(b_in, dtype=np.float32)).reshape(1, E)
    w_out = np.ascontiguousarray(np.asarray(w_out, dtype=np.float32))
    b_out = np.ascontiguousarray(np.asarray(b_out, dtype=np.float32)).reshape(1, C)

    in_maps = []
    for c in range(N_CORES):
        sl = slice(c * BP, (c + 1) * BP)
        in_maps.append({
            "x": x[sl], "te": te[sl], "keys": keys[sl], "values": values[sl],
            "mask": mask[sl], "w_in": w_in, "b_in": b_in, "w_out": w_out,
            "b_out": b_out,
        })

    res = _RUNNER(in_maps)
    out = res["out"].reshape(B, T, C)
    attn = res["attn"].reshape(B, T, S)
    return out, attn

